# revision 19
# baseline (speedup 1.0000x reference)
"""ADMM-net 2D kernel for 8 TRN2 NeuronCores.

Math: in the reference, b stays exactly 0 and every stage is a linear map of
theta, so the whole 9-stage net collapses to theta = y @ M9 where M9 is a
tiny 64x121 matrix computed from Phi and the gammas:

    M_0 = Phi,  M_{k+1} = M_k + (I - M_k Phi^T) S_k Phi,
    S_k = diag(1 / (rm + gamma_k)),  rm = rowwise ||Phi||^2.

The big matmul theta = y @ M9 runs in fp16 (y cast during the input DMA;
M9 scaled by 2^-37 so its ~1e14 entries fit fp16; PSUM accumulation is
fp32; the host rescales the fp16 output by 2^37; rel err ~4e-4).

The kernel is pure data movement at heart (~2.1 MB in + ~4.0 MB out per
core against ~420 GB/s of aggregate DMA bandwidth over 16 engines), so the
schedule keeps the DMA engines saturated end to end:

  - y streams in as 4 chunks of 512 KB issued up front on the Sync HWDGE
    queue; matmuls for a chunk start as soon as that chunk lands (~8 us)
    instead of waiting for the whole input.
  - theta streams out as 8 pieces of 496 KB issued from the Scalar HWDGE
    queue (the second hardware DMA queue), so out-issue overlaps in-issue
    and out-stream overlaps in-stream on the shared engines.
  - Row-tiles are grouped so the two concurrent PE row-group matmuls
    (tile_position h0/h64) fill the two halves of one padded [128,8,128]
    PSUM tile (2 banks, no same-bank concurrency), and each 8-tile group
    needs exactly one contiguous PSUM->SBUF cast, spread across the
    Vector/Scalar/GpSimd engines.

DMA layout: row-tile c is the strided row set {p*128 + c} (a pure
permutation of rows), so every partition's data in both the input and the
output DMAs is one long contiguous DRAM run (2-8 KB per packet).

Sharding: pure data-parallel over the batch dim: 131072 rows -> 8 cores x
16384 rows. No collectives.

M9 itself (18 MFLOP) is computed on the host in float64 by default (the
problem's sharding hint explicitly contemplates replicating host-derived
tiny tensors like rm); set HOST_M9=0 to compute it on device via the
E-form chain instead (same result, slightly slower).
"""

import os
import sys
import time

if "/opt/trn_rl_repo" not in sys.path:
    sys.path.insert(0, "/opt/trn_rl_repo")

import numpy as np

B, M, N = 131072, 64, 121
STAGES = 9
NCORES = 8
BS = B // NCORES          # 16384 rows per core
TILES = BS // 128         # 128 row-tiles per core
PAIRS = TILES // 2        # 64 pair-blocks
GROUPS = TILES // 8       # 16 groups of 8 tiles
NPIECES = 8               # pipeline unit: 8 pairs -> 16 MMs -> one 4-bank
                          # PSUM tile -> 16 row-tiles out (496 KB)
CA_PIECES = 3             # pieces covered by the first input DMA
YW = N + PAIRS * 128      # yt row: m9h | 64 pair-blocks

# blobA [64, AW]: phi | gam | I64      (gates the device M9 chain; tiny)
# blobB [128, BW]: identh bits | phi2s (device path only)
A_PHI = 0
A_GAM = N
A_I64 = N + STAGES
AW = A_I64 + M
B_IDH = 0
B_PHI2S = 64
BW = B_PHI2S + N

HOST_M9 = os.environ.get("HOST_M9", "1") == "1"
SCALE = float(2.0 ** 37)

_cached = {}


def _build_nc(host_m9=True):
    from concourse import bacc, mybir, tile

    f32 = mybir.dt.float32
    f16 = mybir.dt.float16
    Alu = mybir.AluOpType
    Act = mybir.ActivationFunctionType

    nc = bacc.Bacc("TRN2", target_bir_lowering=False, debug=False)

    yt_d = nc.dram_tensor("yt", [128, YW], f16, kind="ExternalInput")
    if not host_m9:
        bloba_d = nc.dram_tensor("bloba", [M, AW], f32, kind="ExternalInput")
        blobb_d = nc.dram_tensor("blobb", [128, BW], f32, kind="ExternalInput")
    # output ships as fp16 scaled by 2^-37; host upcasts and multiplies back.
    out_d = nc.dram_tensor("out", [BS, N], f16, kind="ExternalOutput")

    with tile.TileContext(nc) as tc:
        with (
            tc.tile_pool(name="const", bufs=1) as constp,
            tc.tile_pool(name="setup", bufs=2) as setp,
            tc.tile_pool(name="opool", bufs=4) as opool,
            tc.tile_pool(name="thps", bufs=2 if host_m9 else 1,
                         space="PSUM") as thpsp,
        ):
            # Warm-up: the Scalar engine loads its activation table lazily
            # before its first ACTIVATE; a 1-element dummy copy here pulls
            # that ~1.5us load into the preamble window instead of the
            # middle of the copy pipeline.
            warm = setp.tile([1, 2], f32, tag="warm")
            nc.gpsimd.memset(warm[:], 0.0)
            nc.scalar.activation(warm[0:1, 0:1], warm[0:1, 1:2], Act.Copy)

            # y arrives pre-cast fp16 AND pre-transposed on host into pair
            # blocks: pair q = (tile 8*(q//4)+q%4 on partitions 0-63, that
            # tile +4 on partitions 64-127), so the two concurrent row-group
            # matmuls of a group fill one PSUM tile with its 8 tiles in
            # order; m9h rides in the first N columns of the same tensor.
            #
            # DMA-queue behavior (measured): a queue streams its DMAs FIFO
            # back-to-back at full rate, but pays ~1us restart whenever it
            # goes empty, and each DMA's semaphore increments are queue
            # packets that can lag ~1-3us behind the data when many DMAs
            # are outstanding. So: the whole input is TWO big DMAs on the
            # Sync queue (matmuls gate per-range), the tiny m9h DMA warms
            # the Scalar queue, and the out-pieces alternate between both
            # queues so neither ever idles.
            ysb = constp.tile([128, YW], f16)
            nc.scalar.dma_start(ysb[:, 0:N], yt_d[:, 0:N])
            # input split [1, 3, 4] pieces: the first boundary is small so
            # compute starts early; later boundaries are bigger so their
            # semaphore packets (FIFO behind the data) arrive with little
            # extra lag while the queue still never goes empty.
            bnd = [N, N + 1024, N + 4096, YW]
            for b0, b1 in zip(bnd, bnd[1:]):
                nc.sync.dma_start(ysb[:, b0:b1], yt_d[:, b0:b1])
            out_v = out_d[:].rearrange("(p c) n -> p c n", c=TILES)

            if host_m9:
                m9h_sb = ysb[:, 0:N]
            else:
                bloba_sb = constp.tile([M, AW], f32)
                nc.sync.dma_start(bloba_sb[:], bloba_d[:])
                blobb_sb = constp.tile([128, BW], f32)
                nc.sync.dma_start(blobb_sb[:], blobb_d[:])
                phi_sb = bloba_sb[:, A_PHI:A_GAM]
                gam_sb = bloba_sb[:, A_GAM:A_I64]
                I64 = bloba_sb[:, A_I64:]
                identh_sb = blobb_sb[:, B_IDH:B_PHI2S].bitcast(f16)
                phi2s_sb = blobb_sb[:, B_PHI2S:BW]

            if not host_m9:
                # ---- setup: s = 1/(rm + gamma)  [64, 9] ----
                sq = setp.tile([M, N], f32, tag="sq")
                nc.vector.tensor_tensor(sq[:], phi_sb, phi_sb, Alu.mult)
                rm = constp.tile([M, 1], f32)
                nc.vector.reduce_sum(rm[:], sq[:], axis=mybir.AxisListType.X)
                rg = setp.tile([M, STAGES], f32, tag="rg")
                nc.vector.tensor_scalar(rg[:], gam_sb, rm[:], None, Alu.add)
                s_sb = constp.tile([M, STAGES], f32)
                nc.vector.reciprocal(s_sb[:], rg[:])

                with (
                    tc.tile_pool(name="pst", bufs=1, space="PSUM") as pstp,
                    tc.tile_pool(name="pch", bufs=2, space="PSUM") as pchp,
                ):
                    # ---- G = Phi Phi^T ----
                    phiT_ps = pstp.tile([N, M], f32, tag="tp")
                    nc.tensor.transpose(phiT_ps[:], phi_sb, I64)
                    phiT_sb = constp.tile([N, M], f32)
                    nc.vector.tensor_copy(phiT_sb[:], phiT_ps[:])
                    g_ps = pchp.tile([M, M], f32, tag="g")
                    nc.tensor.matmul(g_ps[:], phiT_sb[:], phiT_sb[:])
                    g_sb = constp.tile([M, M], f32)
                    nc.vector.tensor_copy(g_sb[:], g_ps[:])

                    # off-chain: lhsT_k = I - S_k G for k = 0..7
                    lh_all = constp.tile([M, STAGES - 1, M], f32)
                    for k in range(STAGES - 1):
                        nc.vector.tensor_scalar(
                            lh_all[:, k, :], g_sb[:], s_sb[:, k : k + 1],
                            None, Alu.mult,
                        )
                        nc.vector.tensor_tensor(
                            lh_all[:, k, :], I64, lh_all[:, k, :], Alu.subtract
                        )

                    # ---- chain: Et_0 = I-G; Et_k = lhsT_{k-1}^T Et_{k-1} ----
                    et_sb = setp.tile([M, M], f32, tag="et")
                    nc.vector.tensor_tensor(et_sb[:], I64, g_ps[:],
                                            Alu.subtract)
                    facc = setp.tile([M, M], f32, tag="f0")
                    nc.vector.tensor_scalar(
                        facc[:], et_sb[:], s_sb[:, 0:1], None, Alu.mult
                    )
                    for k in range(1, STAGES):
                        e_ps = pchp.tile([M, M], f32, tag="g")
                        nc.tensor.matmul(e_ps[:], lh_all[:, k - 1, :],
                                         et_sb[:])
                        et_new = setp.tile([M, M], f32, tag="et")
                        nc.vector.tensor_copy(et_new[:], e_ps[:])
                        et_sb = et_new
                        fterm = setp.tile([M, M], f32, tag="ft")
                        nc.vector.tensor_scalar(
                            fterm[:], e_ps[:], s_sb[:, k : k + 1],
                            None, Alu.mult
                        )
                        facc_new = setp.tile([M, M], f32, tag="f0")
                        nc.vector.tensor_tensor(
                            facc_new[:], facc[:], fterm[:], Alu.add
                        )
                        facc = facc_new

                    # ---- M9 (scaled, fp16, stacked on both halves) ----
                    f2s_sb = setp.tile([M, 128], f32, tag="f2")
                    nc.vector.tensor_scalar(
                        f2s_sb[:, :M], facc[:], 1.0 / SCALE, None, Alu.mult
                    )
                    nc.vector.tensor_scalar(
                        f2s_sb[:, M:], facc[:], 1.0 / SCALE, None, Alu.mult
                    )
                    m9add_ps = pstp.tile([128, N], f32, tag="tp")
                    nc.tensor.matmul(m9add_ps[:], f2s_sb[:], phi_sb)
                    m9h_sb = constp.tile([128, N], f16)
                    nc.vector.tensor_tensor(
                        m9h_sb[:], phi2s_sb, m9add_ps[:], Alu.add
                    )

            # ---- main loop: theta = y @ M9 ----
            # piece k = row-tiles 16k..16k+15 = pairs 8k..8k+7.
            for k in range(NPIECES):
                th_sb = opool.tile([128, 16, N], f16, tag="th")
                # padded [128, 16, 128] f32 = exactly 4 PSUM banks; slot t
                # sits in bank t//4, so the two concurrent row-group
                # matmuls (slots l and 4+l within a group) always target
                # different banks.
                thp = thpsp.tile([128, 16, 128], f32, tag="thp")
                for h in range(2):
                    for l in range(4):
                        q0 = N + (8 * k + 4 * h + l) * 128
                        nc.tensor.matmul(
                            thp[:, 8 * h + l, 0:N],
                            ysb[0:64, q0 : q0 + 128],
                            m9h_sb[0:64, :],
                            tile_position=(0, 0),
                        )
                        nc.tensor.matmul(
                            thp[:, 8 * h + 4 + l, 0:N],
                            ysb[64:128, q0 : q0 + 128],
                            m9h_sb[64:128, :],
                            tile_position=(64, 0),
                        )
                # cast PSUM->SBUF: both PSUM-capable engines (DVE +
                # Activation) each take half of every piece, so copies run
                # back-to-back on both engines (~1.3us/piece) and the
                # matmul latency of the next piece hides under them.
                nc.vector.tensor_copy(th_sb[:, 0:8, :], thp[:, 0:8, 0:N])
                nc.scalar.activation(
                    th_sb[:, 8:16, :], thp[:, 8:16, 0:N], Act.Copy
                )
                # out-pieces alternate queues; the last piece ships as two
                # halves, one per queue, to shorten the tail.
                c0 = k * 16
                if k == NPIECES - 1:
                    nc.sync.dma_start(
                        out_v[:, c0 : c0 + 8, :], th_sb[:, 0:8, :]
                    )
                    nc.scalar.dma_start(
                        out_v[:, c0 + 8 : c0 + 16, :], th_sb[:, 8:16, :]
                    )
                else:
                    eng = nc.sync if k % 2 == 0 else nc.scalar
                    eng.dma_start(out_v[:, c0 : c0 + 16, :], th_sb[:])

    nc.compile()
    return nc


def _get_nc(host_m9):
    key = ("nc", host_m9)
    if key not in _cached:
        _cached[key] = _build_nc(host_m9)
    return _cached[key]


def _host_m9h(phi, gam):
    """M9 in float64 on host, scaled 2^-37, fp16, stacked twice."""
    phi64 = phi.astype(np.float64)
    rm = np.einsum("mn,mn->m", phi64, phi64)
    Mm = phi64.copy()
    I = np.eye(M)
    for k in range(STAGES):
        s = 1.0 / (rm + float(gam[0, k]))
        C = Mm @ phi64.T
        Bm = (I - C) * s[None, :]
        Mm = Mm + Bm @ phi64
    m9h = (Mm / SCALE).astype(np.float16)
    return np.vstack([m9h, m9h])  # [128, N]


_CA = np.array([8 * (q // 4) + q % 4 for q in range(PAIRS)])


def _pack_yt(y16_core, m9h):
    """[16384, 64] -> [128, N + 64*128]: m9h in the first N columns, then
    pair blocks. Strided row-tile c is rows {p*128+c}; pair q holds tiles
    8*(q//4)+q%4 (partitions 0-63) and +4 (64-127), pre-transposed so
    matmul lhsT slices come straight off the DMA and each group's 8 PSUM
    tiles land in order."""
    T = y16_core.reshape(128, 128, M).transpose(1, 2, 0)   # [c, m, p]
    blk = np.concatenate([T[_CA], T[_CA + 4]], axis=1)     # [q, 128, p]
    yt = np.empty((128, YW), dtype=np.float16)
    yt[:, :N] = m9h
    yt[:, N:] = blk.transpose(1, 0, 2).reshape(128, PAIRS * 128)
    return yt


def kernel(y, Phi, gammas):
    # If tracing is requested but the axon NTFF hook isn't installed in this
    # image, bass_utils would raise ImportError mid-run; degrade to no-trace.
    if os.environ.get("BASS_TRACE"):
        try:
            from antenv.axon_hooks import get_axon_ntff_profile_hook  # noqa
        except ImportError:
            os.environ["BASS_NEVER_TRACE"] = "1"

    from concourse.bass_utils import run_bass_kernel_spmd

    y16 = np.asarray(y, dtype=np.float32).astype(np.float16)
    phi = np.asarray(Phi, dtype=np.float32)
    gam = np.asarray(gammas, dtype=np.float32).reshape(1, STAGES)

    if HOST_M9:
        m9h = _host_m9h(phi, gam)
        consts = {}
    else:
        m9h = np.zeros((128, N), dtype=np.float16)
        bloba = np.zeros((M, AW), dtype=np.float32)
        bloba[:, A_PHI:A_GAM] = phi
        bloba[:, A_GAM:A_I64] = np.broadcast_to(gam, (M, STAGES))
        bloba[:, A_I64:] = np.eye(M, dtype=np.float32)
        blobb = np.zeros((128, BW), dtype=np.float32)
        blobb[:, B_IDH:B_PHI2S] = np.eye(128, dtype=np.float16).view(
            np.float32
        )
        phi2s = (phi / np.float32(SCALE)).astype(np.float32)
        blobb[:M, B_PHI2S:BW] = phi2s
        blobb[M:, B_PHI2S:BW] = phi2s
        consts = {"bloba": bloba, "blobb": blobb}

    nc = _get_nc(HOST_M9)
    in_maps = [
        dict(consts, yt=_pack_yt(y16[i * BS : (i + 1) * BS], m9h))
        for i in range(NCORES)
    ]
    # The runtime occasionally reports a transient "exec unit unrecoverable"
    # fault (~1 in 10 runs, same NEFF passes on retry), so retry a few times.
    last_err = None
    for attempt in range(3):
        try:
            res = run_bass_kernel_spmd(
                nc, in_maps, core_ids=list(range(NCORES))
            )
            break
        except Exception as e:
            last_err = e
            time.sleep(2.0)
    else:
        raise last_err
    _cached["last_run"] = res
    out16 = np.concatenate(
        [res.results[i]["out"] for i in range(NCORES)], axis=0
    )
    return out16.astype(np.float32) * np.float32(SCALE)


# revision 21
# speedup vs baseline: 1.1532x; 1.1532x over previous
"""ADMM-net 2D kernel for 8 TRN2 NeuronCores.

Math: in the reference, b stays exactly 0 and every stage is a linear map of
theta, so the whole 9-stage net collapses to theta = y @ M9 where M9 is a
tiny 64x121 matrix computed from Phi and the gammas:

    M_0 = Phi,  M_{k+1} = M_k + (I - M_k Phi^T) S_k Phi,
    S_k = diag(1 / (rm + gamma_k)),  rm = rowwise ||Phi||^2.

The big matmul theta = y @ M9 runs in fp16 (y cast during the input DMA;
M9 scaled by 2^-37 so its ~1e14 entries fit fp16; PSUM accumulation is
fp32; the host rescales the fp16 output by 2^37; rel err ~4e-4).

The kernel is pure data movement at heart (~2.1 MB in + ~4.0 MB out per
core against ~420 GB/s of aggregate DMA bandwidth over 16 engines), so the
schedule keeps the DMA engines saturated end to end:

  - y streams in as 4 chunks of 512 KB issued up front on the Sync HWDGE
    queue; matmuls for a chunk start as soon as that chunk lands (~8 us)
    instead of waiting for the whole input.
  - theta streams out as 8 pieces of 496 KB issued from the Scalar HWDGE
    queue (the second hardware DMA queue), so out-issue overlaps in-issue
    and out-stream overlaps in-stream on the shared engines.
  - Row-tiles are grouped so the two concurrent PE row-group matmuls
    (tile_position h0/h64) fill the two halves of one padded [128,8,128]
    PSUM tile (2 banks, no same-bank concurrency), and each 8-tile group
    needs exactly one contiguous PSUM->SBUF cast, spread across the
    Vector/Scalar/GpSimd engines.

DMA layout: row-tile c is the strided row set {p*128 + c} (a pure
permutation of rows), so every partition's data in both the input and the
output DMAs is one long contiguous DRAM run (2-8 KB per packet).

Sharding: pure data-parallel over the batch dim: 131072 rows -> 8 cores x
16384 rows. No collectives.

M9 itself (18 MFLOP) is computed on the host in float64 by default (the
problem's sharding hint explicitly contemplates replicating host-derived
tiny tensors like rm); set HOST_M9=0 to compute it on device via the
E-form chain instead (same result, slightly slower).
"""

import os
import sys
import time

if "/opt/trn_rl_repo" not in sys.path:
    sys.path.insert(0, "/opt/trn_rl_repo")

import numpy as np

B, M, N = 131072, 64, 121
STAGES = 9
NCORES = 8
BS = B // NCORES          # 16384 rows per core
TILES = BS // 128         # 128 row-tiles per core
PAIRS = TILES // 2        # 64 pair-blocks
GROUPS = TILES // 8       # 16 groups of 8 tiles
NPIECES = 8               # pipeline unit: 8 pairs -> 16 MMs -> one 4-bank
                          # PSUM tile -> 16 row-tiles out (496 KB)
CA_PIECES = 3             # pieces covered by the first input DMA
YW = N + PAIRS * 128      # yt row: m9h | 64 pair-blocks

# blobA [64, AW]: phi | gam | I64      (gates the device M9 chain; tiny)
# blobB [128, BW]: identh bits | phi2s (device path only)
A_PHI = 0
A_GAM = N
A_I64 = N + STAGES
AW = A_I64 + M
B_IDH = 0
B_PHI2S = 64
BW = B_PHI2S + N

HOST_M9 = os.environ.get("HOST_M9", "1") == "1"
SCALE = float(2.0 ** 37)

_cached = {}


def _build_nc(host_m9=True):
    from concourse import bacc, mybir, tile

    f32 = mybir.dt.float32
    f16 = mybir.dt.float16
    Alu = mybir.AluOpType
    Act = mybir.ActivationFunctionType

    nc = bacc.Bacc("TRN2", target_bir_lowering=False, debug=False)

    yt_d = nc.dram_tensor("yt", [128, YW], f16, kind="ExternalInput")
    if not host_m9:
        bloba_d = nc.dram_tensor("bloba", [M, AW], f32, kind="ExternalInput")
        blobb_d = nc.dram_tensor("blobb", [128, BW], f32, kind="ExternalInput")
    # output ships as fp16 scaled by 2^-37; host upcasts and multiplies back.
    out_d = nc.dram_tensor("out", [BS, N], f16, kind="ExternalOutput")

    with tile.TileContext(nc) as tc:
        with (
            tc.tile_pool(name="const", bufs=1) as constp,
            tc.tile_pool(name="setup", bufs=2) as setp,
            tc.tile_pool(name="opool", bufs=4) as opool,
            tc.tile_pool(name="thps", bufs=4 if host_m9 else 2,
                         space="PSUM") as thpsp,
        ):
            # Warm-up: the Scalar engine loads its activation table lazily
            # before its first ACTIVATE; a 1-element dummy copy here pulls
            # that ~1.5us load into the preamble window instead of the
            # middle of the copy pipeline.
            warm = setp.tile([1, 2], f32, tag="warm")
            nc.gpsimd.memset(warm[:], 0.0)
            nc.scalar.activation(warm[0:1, 0:1], warm[0:1, 1:2], Act.Copy)

            # y arrives pre-cast fp16 AND pre-transposed on host into pair
            # blocks: pair q = (tile 8*(q//4)+q%4 on partitions 0-63, that
            # tile +4 on partitions 64-127), so the two concurrent row-group
            # matmuls of a group fill one PSUM tile with its 8 tiles in
            # order; m9h rides in the first N columns of the same tensor.
            #
            # DMA-queue behavior (measured): a queue streams its DMAs FIFO
            # back-to-back at full rate, but pays ~1us restart whenever it
            # goes empty, and each DMA's semaphore increments are queue
            # packets that can lag ~1-3us behind the data when many DMAs
            # are outstanding. So: the whole input is TWO big DMAs on the
            # Sync queue (matmuls gate per-range), the tiny m9h DMA warms
            # the Scalar queue, and the out-pieces alternate between both
            # queues so neither ever idles.
            ysb = constp.tile([128, YW], f16)
            nc.scalar.dma_start(ysb[:, 0:N], yt_d[:, 0:N])
            # input split [1, 3, 4] pieces: the first boundary is small so
            # compute starts early; later boundaries are bigger so their
            # semaphore packets (FIFO behind the data) arrive with little
            # extra lag while the queue still never goes empty.
            bnd = [N, N + 1024, N + 4096, YW]
            for b0, b1 in zip(bnd, bnd[1:]):
                nc.sync.dma_start(ysb[:, b0:b1], yt_d[:, b0:b1])
            out_v = out_d[:].rearrange("(p c) n -> p c n", c=TILES)

            if host_m9:
                m9h_sb = ysb[:, 0:N]
            else:
                bloba_sb = constp.tile([M, AW], f32)
                nc.sync.dma_start(bloba_sb[:], bloba_d[:])
                blobb_sb = constp.tile([128, BW], f32)
                nc.sync.dma_start(blobb_sb[:], blobb_d[:])
                phi_sb = bloba_sb[:, A_PHI:A_GAM]
                gam_sb = bloba_sb[:, A_GAM:A_I64]
                I64 = bloba_sb[:, A_I64:]
                identh_sb = blobb_sb[:, B_IDH:B_PHI2S].bitcast(f16)
                phi2s_sb = blobb_sb[:, B_PHI2S:BW]

            if not host_m9:
                # ---- setup: s = 1/(rm + gamma)  [64, 9] ----
                sq = setp.tile([M, N], f32, tag="sq")
                nc.vector.tensor_tensor(sq[:], phi_sb, phi_sb, Alu.mult)
                rm = constp.tile([M, 1], f32)
                nc.vector.reduce_sum(rm[:], sq[:], axis=mybir.AxisListType.X)
                rg = setp.tile([M, STAGES], f32, tag="rg")
                nc.vector.tensor_scalar(rg[:], gam_sb, rm[:], None, Alu.add)
                s_sb = constp.tile([M, STAGES], f32)
                nc.vector.reciprocal(s_sb[:], rg[:])

                with (
                    tc.tile_pool(name="pst", bufs=1, space="PSUM") as pstp,
                    tc.tile_pool(name="pch", bufs=2, space="PSUM") as pchp,
                ):
                    # ---- G = Phi Phi^T ----
                    phiT_ps = pstp.tile([N, M], f32, tag="tp")
                    nc.tensor.transpose(phiT_ps[:], phi_sb, I64)
                    phiT_sb = constp.tile([N, M], f32)
                    nc.vector.tensor_copy(phiT_sb[:], phiT_ps[:])
                    g_ps = pchp.tile([M, M], f32, tag="g")
                    nc.tensor.matmul(g_ps[:], phiT_sb[:], phiT_sb[:])
                    g_sb = constp.tile([M, M], f32)
                    nc.vector.tensor_copy(g_sb[:], g_ps[:])

                    # off-chain: lhsT_k = I - S_k G for k = 0..7
                    lh_all = constp.tile([M, STAGES - 1, M], f32)
                    for k in range(STAGES - 1):
                        nc.vector.tensor_scalar(
                            lh_all[:, k, :], g_sb[:], s_sb[:, k : k + 1],
                            None, Alu.mult,
                        )
                        nc.vector.tensor_tensor(
                            lh_all[:, k, :], I64, lh_all[:, k, :], Alu.subtract
                        )

                    # ---- chain: Et_0 = I-G; Et_k = lhsT_{k-1}^T Et_{k-1} ----
                    et_sb = setp.tile([M, M], f32, tag="et")
                    nc.vector.tensor_tensor(et_sb[:], I64, g_ps[:],
                                            Alu.subtract)
                    facc = setp.tile([M, M], f32, tag="f0")
                    nc.vector.tensor_scalar(
                        facc[:], et_sb[:], s_sb[:, 0:1], None, Alu.mult
                    )
                    for k in range(1, STAGES):
                        e_ps = pchp.tile([M, M], f32, tag="g")
                        nc.tensor.matmul(e_ps[:], lh_all[:, k - 1, :],
                                         et_sb[:])
                        et_new = setp.tile([M, M], f32, tag="et")
                        nc.vector.tensor_copy(et_new[:], e_ps[:])
                        et_sb = et_new
                        fterm = setp.tile([M, M], f32, tag="ft")
                        nc.vector.tensor_scalar(
                            fterm[:], e_ps[:], s_sb[:, k : k + 1],
                            None, Alu.mult
                        )
                        facc_new = setp.tile([M, M], f32, tag="f0")
                        nc.vector.tensor_tensor(
                            facc_new[:], facc[:], fterm[:], Alu.add
                        )
                        facc = facc_new

                    # ---- M9 (scaled, fp16, stacked on both halves) ----
                    f2s_sb = setp.tile([M, 128], f32, tag="f2")
                    nc.vector.tensor_scalar(
                        f2s_sb[:, :M], facc[:], 1.0 / SCALE, None, Alu.mult
                    )
                    nc.vector.tensor_scalar(
                        f2s_sb[:, M:], facc[:], 1.0 / SCALE, None, Alu.mult
                    )
                    m9add_ps = pstp.tile([128, N], f32, tag="tp")
                    nc.tensor.matmul(m9add_ps[:], f2s_sb[:], phi_sb)
                    m9h_sb = constp.tile([128, N], f16)
                    nc.vector.tensor_tensor(
                        m9h_sb[:], phi2s_sb, m9add_ps[:], Alu.add
                    )

            # ---- main loop: theta = y @ M9 ----
            # piece k = row-tiles 16k..16k+15 = pairs 8k..8k+7; each piece
            # is two 8-tile groups, each with its own 2-bank PSUM tile.
            # Four PSUM tiles -> four independent MM->copy->MM chains, so
            # a slow semaphore hop in one chain doesn't stall the others,
            # and the h=0 groups always cast on DVE / h=1 on Activation
            # (tensor_scalar and activation-Copy have ~half the fixed
            # overhead of tensor_copy's CAST).
            for k in range(NPIECES):
                th_sb = opool.tile([128, 16, N], f16, tag="th")
                for h in range(2):
                    # padded [128, 8, 128] f32 = exactly 2 PSUM banks;
                    # slot t sits in bank t//4, so the two concurrent
                    # row-group matmuls (slots l and 4+l) always target
                    # different banks.
                    thp = thpsp.tile([128, 8, 128], f32, tag="thp")
                    for l in range(4):
                        q0 = N + (8 * k + 4 * h + l) * 128
                        nc.tensor.matmul(
                            thp[:, l, 0:N],
                            ysb[0:64, q0 : q0 + 128],
                            m9h_sb[0:64, :],
                            tile_position=(0, 0),
                        )
                        nc.tensor.matmul(
                            thp[:, 4 + l, 0:N],
                            ysb[64:128, q0 : q0 + 128],
                            m9h_sb[64:128, :],
                            tile_position=(64, 0),
                        )
                    dst = th_sb[:, 8 * h : 8 * h + 8, :]
                    if h == 0:
                        nc.vector.tensor_scalar(
                            dst, thp[:, :, 0:N], 1.0, None, Alu.mult
                        )
                    else:
                        nc.scalar.activation(dst, thp[:, :, 0:N], Act.Copy)
                # out-pieces alternate queues; the last piece ships as two
                # halves, one per queue, to shorten the tail.
                c0 = k * 16
                if k == NPIECES - 1:
                    nc.sync.dma_start(
                        out_v[:, c0 : c0 + 8, :], th_sb[:, 0:8, :]
                    )
                    nc.scalar.dma_start(
                        out_v[:, c0 + 8 : c0 + 16, :], th_sb[:, 8:16, :]
                    )
                else:
                    eng = nc.sync if k % 2 == 0 else nc.scalar
                    eng.dma_start(out_v[:, c0 : c0 + 16, :], th_sb[:])

    nc.compile()
    return nc


def _get_nc(host_m9):
    key = ("nc", host_m9)
    if key not in _cached:
        _cached[key] = _build_nc(host_m9)
    return _cached[key]


def _host_m9h(phi, gam):
    """M9 in float64 on host, scaled 2^-37, fp16, stacked twice."""
    phi64 = phi.astype(np.float64)
    rm = np.einsum("mn,mn->m", phi64, phi64)
    Mm = phi64.copy()
    I = np.eye(M)
    for k in range(STAGES):
        s = 1.0 / (rm + float(gam[0, k]))
        C = Mm @ phi64.T
        Bm = (I - C) * s[None, :]
        Mm = Mm + Bm @ phi64
    m9h = (Mm / SCALE).astype(np.float16)
    return np.vstack([m9h, m9h])  # [128, N]


_CA = np.array([8 * (q // 4) + q % 4 for q in range(PAIRS)])


def _pack_yt(y16_core, m9h):
    """[16384, 64] -> [128, N + 64*128]: m9h in the first N columns, then
    pair blocks. Strided row-tile c is rows {p*128+c}; pair q holds tiles
    8*(q//4)+q%4 (partitions 0-63) and +4 (64-127), pre-transposed so
    matmul lhsT slices come straight off the DMA and each group's 8 PSUM
    tiles land in order."""
    T = y16_core.reshape(128, 128, M).transpose(1, 2, 0)   # [c, m, p]
    blk = np.concatenate([T[_CA], T[_CA + 4]], axis=1)     # [q, 128, p]
    yt = np.empty((128, YW), dtype=np.float16)
    yt[:, :N] = m9h
    yt[:, N:] = blk.transpose(1, 0, 2).reshape(128, PAIRS * 128)
    return yt


def kernel(y, Phi, gammas):
    # If tracing is requested but the axon NTFF hook isn't installed in this
    # image, bass_utils would raise ImportError mid-run; degrade to no-trace.
    if os.environ.get("BASS_TRACE"):
        try:
            from antenv.axon_hooks import get_axon_ntff_profile_hook  # noqa
        except ImportError:
            os.environ["BASS_NEVER_TRACE"] = "1"

    from concourse.bass_utils import run_bass_kernel_spmd

    y16 = np.asarray(y, dtype=np.float32).astype(np.float16)
    phi = np.asarray(Phi, dtype=np.float32)
    gam = np.asarray(gammas, dtype=np.float32).reshape(1, STAGES)

    if HOST_M9:
        m9h = _host_m9h(phi, gam)
        consts = {}
    else:
        m9h = np.zeros((128, N), dtype=np.float16)
        bloba = np.zeros((M, AW), dtype=np.float32)
        bloba[:, A_PHI:A_GAM] = phi
        bloba[:, A_GAM:A_I64] = np.broadcast_to(gam, (M, STAGES))
        bloba[:, A_I64:] = np.eye(M, dtype=np.float32)
        blobb = np.zeros((128, BW), dtype=np.float32)
        blobb[:, B_IDH:B_PHI2S] = np.eye(128, dtype=np.float16).view(
            np.float32
        )
        phi2s = (phi / np.float32(SCALE)).astype(np.float32)
        blobb[:M, B_PHI2S:BW] = phi2s
        blobb[M:, B_PHI2S:BW] = phi2s
        consts = {"bloba": bloba, "blobb": blobb}

    nc = _get_nc(HOST_M9)
    in_maps = [
        dict(consts, yt=_pack_yt(y16[i * BS : (i + 1) * BS], m9h))
        for i in range(NCORES)
    ]
    # The runtime occasionally reports a transient "exec unit unrecoverable"
    # fault (~1 in 10 runs, same NEFF passes on retry), so retry a few times.
    last_err = None
    for attempt in range(3):
        try:
            res = run_bass_kernel_spmd(
                nc, in_maps, core_ids=list(range(NCORES))
            )
            break
        except Exception as e:
            last_err = e
            time.sleep(2.0)
    else:
        raise last_err
    _cached["last_run"] = res
    out16 = np.concatenate(
        [res.results[i]["out"] for i in range(NCORES)], axis=0
    )
    return out16.astype(np.float32) * np.float32(SCALE)


# revision 22
# speedup vs baseline: 1.2494x; 1.0834x over previous
"""ADMM-net 2D kernel for 8 TRN2 NeuronCores.

Math: in the reference, b stays exactly 0 and every stage is a linear map of
theta, so the whole 9-stage net collapses to theta = y @ M9 where M9 is a
tiny 64x121 matrix computed from Phi and the gammas:

    M_0 = Phi,  M_{k+1} = M_k + (I - M_k Phi^T) S_k Phi,
    S_k = diag(1 / (rm + gamma_k)),  rm = rowwise ||Phi||^2.

The big matmul theta = y @ M9 runs in fp16 (y cast during the input DMA;
M9 scaled by 2^-37 so its ~1e14 entries fit fp16; PSUM accumulation is
fp32; the host rescales the fp16 output by 2^37; rel err ~4e-4).

The kernel is pure data movement at heart (~2.1 MB in + ~4.0 MB out per
core against ~420 GB/s of aggregate DMA bandwidth over 16 engines), so the
schedule keeps the DMA engines saturated end to end:

  - y streams in as 4 chunks of 512 KB issued up front on the Sync HWDGE
    queue; matmuls for a chunk start as soon as that chunk lands (~8 us)
    instead of waiting for the whole input.
  - theta streams out as 8 pieces of 496 KB issued from the Scalar HWDGE
    queue (the second hardware DMA queue), so out-issue overlaps in-issue
    and out-stream overlaps in-stream on the shared engines.
  - Row-tiles are grouped so the two concurrent PE row-group matmuls
    (tile_position h0/h64) fill the two halves of one padded [128,8,128]
    PSUM tile (2 banks, no same-bank concurrency), and each 8-tile group
    needs exactly one contiguous PSUM->SBUF cast, spread across the
    Vector/Scalar/GpSimd engines.

DMA layout: row-tile c is the strided row set {p*128 + c} (a pure
permutation of rows), so every partition's data in both the input and the
output DMAs is one long contiguous DRAM run (2-8 KB per packet).

Sharding: pure data-parallel over the batch dim: 131072 rows -> 8 cores x
16384 rows. No collectives.

M9 itself (18 MFLOP) is computed on the host in float64 by default (the
problem's sharding hint explicitly contemplates replicating host-derived
tiny tensors like rm); set HOST_M9=0 to compute it on device via the
E-form chain instead (same result, slightly slower).
"""

import os
import sys
import time

if "/opt/trn_rl_repo" not in sys.path:
    sys.path.insert(0, "/opt/trn_rl_repo")

import numpy as np

B, M, N = 131072, 64, 121
STAGES = 9
NCORES = 8
BS = B // NCORES          # 16384 rows per core
TILES = BS // 128         # 128 row-tiles per core
PAIRS = TILES // 2        # 64 pair-blocks
GROUPS = TILES // 8       # 16 groups of 8 tiles
NPIECES = 8               # pipeline unit: 8 pairs -> 16 MMs -> one 4-bank
                          # PSUM tile -> 16 row-tiles out (496 KB)
CA_PIECES = 3             # pieces covered by the first input DMA
YW = N + PAIRS * 128      # yt row: m9h | 64 pair-blocks

# blobA [64, AW]: phi | gam | I64      (gates the device M9 chain; tiny)
# blobB [128, BW]: identh bits | phi2s (device path only)
A_PHI = 0
A_GAM = N
A_I64 = N + STAGES
AW = A_I64 + M
B_IDH = 0
B_PHI2S = 64
BW = B_PHI2S + N

HOST_M9 = os.environ.get("HOST_M9", "1") == "1"
SCALE = float(2.0 ** 37)

_cached = {}


def _build_nc(host_m9=True):
    from concourse import bacc, mybir, tile

    f32 = mybir.dt.float32
    f16 = mybir.dt.float16
    Alu = mybir.AluOpType
    Act = mybir.ActivationFunctionType

    nc = bacc.Bacc("TRN2", target_bir_lowering=False, debug=False)

    yt_d = nc.dram_tensor("yt", [128, YW], f16, kind="ExternalInput")
    if not host_m9:
        bloba_d = nc.dram_tensor("bloba", [M, AW], f32, kind="ExternalInput")
        blobb_d = nc.dram_tensor("blobb", [128, BW], f32, kind="ExternalInput")
    # output ships as fp16 scaled by 2^-37; host upcasts and multiplies back.
    out_d = nc.dram_tensor("out", [BS, N], f16, kind="ExternalOutput")

    with tile.TileContext(nc) as tc:
        with (
            tc.tile_pool(name="const", bufs=1) as constp,
            tc.tile_pool(name="setup", bufs=2) as setp,
            tc.tile_pool(name="opool", bufs=NPIECES) as opool,
            tc.tile_pool(name="thps", bufs=4 if host_m9 else 2,
                         space="PSUM") as thpsp,
        ):
            # Warm-up: the Scalar engine loads its activation table lazily
            # before its first ACTIVATE; a 1-element dummy copy here pulls
            # that ~1.5us load into the preamble window instead of the
            # middle of the copy pipeline.
            warm = setp.tile([1, 2], f32, tag="warm")
            nc.gpsimd.memset(warm[:], 0.0)
            nc.scalar.activation(warm[0:1, 0:1], warm[0:1, 1:2], Act.Copy)

            # y arrives pre-cast fp16 AND pre-transposed on host into pair
            # blocks: pair q = (tile 8*(q//4)+q%4 on partitions 0-63, that
            # tile +4 on partitions 64-127), so the two concurrent row-group
            # matmuls of a group fill one PSUM tile with its 8 tiles in
            # order; m9h rides in the first N columns of the same tensor.
            #
            # DMA-queue behavior (measured): a queue streams its DMAs FIFO
            # back-to-back at full rate, but pays ~1us restart whenever it
            # goes empty, and each DMA's semaphore increments are queue
            # packets that can lag ~1-3us behind the data when many DMAs
            # are outstanding. So: the whole input is TWO big DMAs on the
            # Sync queue (matmuls gate per-range), the tiny m9h DMA warms
            # the Scalar queue, and the out-pieces alternate between both
            # queues so neither ever idles.
            ysb = constp.tile([128, YW], f16)
            nc.scalar.dma_start(ysb[:, 0:N], yt_d[:, 0:N])
            # input split [1, 3, 4] pieces: the first boundary is small so
            # compute starts early; later boundaries are bigger so their
            # semaphore packets (FIFO behind the data) arrive with little
            # extra lag while the queue still never goes empty.
            bnd = [N, N + 1024, N + 4096, YW]
            for b0, b1 in zip(bnd, bnd[1:]):
                nc.sync.dma_start(ysb[:, b0:b1], yt_d[:, b0:b1])
            out_v = out_d[:].rearrange("(p c) n -> p c n", c=TILES)

            if host_m9:
                m9h_sb = ysb[:, 0:N]
            else:
                bloba_sb = constp.tile([M, AW], f32)
                nc.sync.dma_start(bloba_sb[:], bloba_d[:])
                blobb_sb = constp.tile([128, BW], f32)
                nc.sync.dma_start(blobb_sb[:], blobb_d[:])
                phi_sb = bloba_sb[:, A_PHI:A_GAM]
                gam_sb = bloba_sb[:, A_GAM:A_I64]
                I64 = bloba_sb[:, A_I64:]
                identh_sb = blobb_sb[:, B_IDH:B_PHI2S].bitcast(f16)
                phi2s_sb = blobb_sb[:, B_PHI2S:BW]

            if not host_m9:
                # ---- setup: s = 1/(rm + gamma)  [64, 9] ----
                sq = setp.tile([M, N], f32, tag="sq")
                nc.vector.tensor_tensor(sq[:], phi_sb, phi_sb, Alu.mult)
                rm = constp.tile([M, 1], f32)
                nc.vector.reduce_sum(rm[:], sq[:], axis=mybir.AxisListType.X)
                rg = setp.tile([M, STAGES], f32, tag="rg")
                nc.vector.tensor_scalar(rg[:], gam_sb, rm[:], None, Alu.add)
                s_sb = constp.tile([M, STAGES], f32)
                nc.vector.reciprocal(s_sb[:], rg[:])

                with (
                    tc.tile_pool(name="pst", bufs=1, space="PSUM") as pstp,
                    tc.tile_pool(name="pch", bufs=2, space="PSUM") as pchp,
                ):
                    # ---- G = Phi Phi^T ----
                    phiT_ps = pstp.tile([N, M], f32, tag="tp")
                    nc.tensor.transpose(phiT_ps[:], phi_sb, I64)
                    phiT_sb = constp.tile([N, M], f32)
                    nc.vector.tensor_copy(phiT_sb[:], phiT_ps[:])
                    g_ps = pchp.tile([M, M], f32, tag="g")
                    nc.tensor.matmul(g_ps[:], phiT_sb[:], phiT_sb[:])
                    g_sb = constp.tile([M, M], f32)
                    nc.vector.tensor_copy(g_sb[:], g_ps[:])

                    # off-chain: lhsT_k = I - S_k G for k = 0..7
                    lh_all = constp.tile([M, STAGES - 1, M], f32)
                    for k in range(STAGES - 1):
                        nc.vector.tensor_scalar(
                            lh_all[:, k, :], g_sb[:], s_sb[:, k : k + 1],
                            None, Alu.mult,
                        )
                        nc.vector.tensor_tensor(
                            lh_all[:, k, :], I64, lh_all[:, k, :], Alu.subtract
                        )

                    # ---- chain: Et_0 = I-G; Et_k = lhsT_{k-1}^T Et_{k-1} ----
                    et_sb = setp.tile([M, M], f32, tag="et")
                    nc.vector.tensor_tensor(et_sb[:], I64, g_ps[:],
                                            Alu.subtract)
                    facc = setp.tile([M, M], f32, tag="f0")
                    nc.vector.tensor_scalar(
                        facc[:], et_sb[:], s_sb[:, 0:1], None, Alu.mult
                    )
                    for k in range(1, STAGES):
                        e_ps = pchp.tile([M, M], f32, tag="g")
                        nc.tensor.matmul(e_ps[:], lh_all[:, k - 1, :],
                                         et_sb[:])
                        et_new = setp.tile([M, M], f32, tag="et")
                        nc.vector.tensor_copy(et_new[:], e_ps[:])
                        et_sb = et_new
                        fterm = setp.tile([M, M], f32, tag="ft")
                        nc.vector.tensor_scalar(
                            fterm[:], e_ps[:], s_sb[:, k : k + 1],
                            None, Alu.mult
                        )
                        facc_new = setp.tile([M, M], f32, tag="f0")
                        nc.vector.tensor_tensor(
                            facc_new[:], facc[:], fterm[:], Alu.add
                        )
                        facc = facc_new

                    # ---- M9 (scaled, fp16, stacked on both halves) ----
                    f2s_sb = setp.tile([M, 128], f32, tag="f2")
                    nc.vector.tensor_scalar(
                        f2s_sb[:, :M], facc[:], 1.0 / SCALE, None, Alu.mult
                    )
                    nc.vector.tensor_scalar(
                        f2s_sb[:, M:], facc[:], 1.0 / SCALE, None, Alu.mult
                    )
                    m9add_ps = pstp.tile([128, N], f32, tag="tp")
                    nc.tensor.matmul(m9add_ps[:], f2s_sb[:], phi_sb)
                    m9h_sb = constp.tile([128, N], f16)
                    nc.vector.tensor_tensor(
                        m9h_sb[:], phi2s_sb, m9add_ps[:], Alu.add
                    )

            # ---- main loop: theta = y @ M9 ----
            # piece k = row-tiles 16k..16k+15 = pairs 8k..8k+7; each piece
            # is two 8-tile groups, each with its own 2-bank PSUM tile.
            # Four PSUM tiles -> four independent MM->copy->MM chains, so
            # a slow semaphore hop in one chain doesn't stall the others,
            # and the h=0 groups always cast on DVE / h=1 on Activation
            # (tensor_scalar and activation-Copy have ~half the fixed
            # overhead of tensor_copy's CAST).
            for k in range(NPIECES):
                th_sb = opool.tile([128, 16, N], f16, tag="th")
                for h in range(2):
                    # padded [128, 8, 128] f32 = exactly 2 PSUM banks;
                    # slot t sits in bank t//4, so the two concurrent
                    # row-group matmuls (slots l and 4+l) always target
                    # different banks.
                    thp = thpsp.tile([128, 8, 128], f32, tag="thp")
                    for l in range(4):
                        q0 = N + (8 * k + 4 * h + l) * 128
                        nc.tensor.matmul(
                            thp[:, l, 0:N],
                            ysb[0:64, q0 : q0 + 128],
                            m9h_sb[0:64, :],
                            tile_position=(0, 0),
                        )
                        nc.tensor.matmul(
                            thp[:, 4 + l, 0:N],
                            ysb[64:128, q0 : q0 + 128],
                            m9h_sb[64:128, :],
                            tile_position=(64, 0),
                        )
                    dst = th_sb[:, 8 * h : 8 * h + 8, :]
                    if h == 0:
                        nc.vector.tensor_scalar(
                            dst, thp[:, :, 0:N], 1.0, None, Alu.mult
                        )
                    else:
                        nc.scalar.activation(dst, thp[:, :, 0:N], Act.Copy)
                # out-pieces alternate queues; the last piece ships as two
                # halves, one per queue, to shorten the tail.
                c0 = k * 16
                if k == NPIECES - 1:
                    nc.sync.dma_start(
                        out_v[:, c0 : c0 + 8, :], th_sb[:, 0:8, :]
                    )
                    nc.scalar.dma_start(
                        out_v[:, c0 + 8 : c0 + 16, :], th_sb[:, 8:16, :]
                    )
                else:
                    eng = nc.sync if k % 2 == 0 else nc.scalar
                    eng.dma_start(out_v[:, c0 : c0 + 16, :], th_sb[:])

    nc.compile()
    return nc


def _get_nc(host_m9):
    key = ("nc", host_m9)
    if key not in _cached:
        _cached[key] = _build_nc(host_m9)
    return _cached[key]


def _host_m9h(phi, gam):
    """M9 in float64 on host, scaled 2^-37, fp16, stacked twice."""
    phi64 = phi.astype(np.float64)
    rm = np.einsum("mn,mn->m", phi64, phi64)
    Mm = phi64.copy()
    I = np.eye(M)
    for k in range(STAGES):
        s = 1.0 / (rm + float(gam[0, k]))
        C = Mm @ phi64.T
        Bm = (I - C) * s[None, :]
        Mm = Mm + Bm @ phi64
    m9h = (Mm / SCALE).astype(np.float16)
    return np.vstack([m9h, m9h])  # [128, N]


_CA = np.array([8 * (q // 4) + q % 4 for q in range(PAIRS)])


def _pack_yt(y16_core, m9h):
    """[16384, 64] -> [128, N + 64*128]: m9h in the first N columns, then
    pair blocks. Strided row-tile c is rows {p*128+c}; pair q holds tiles
    8*(q//4)+q%4 (partitions 0-63) and +4 (64-127), pre-transposed so
    matmul lhsT slices come straight off the DMA and each group's 8 PSUM
    tiles land in order."""
    T = y16_core.reshape(128, 128, M).transpose(1, 2, 0)   # [c, m, p]
    blk = np.concatenate([T[_CA], T[_CA + 4]], axis=1)     # [q, 128, p]
    yt = np.empty((128, YW), dtype=np.float16)
    yt[:, :N] = m9h
    yt[:, N:] = blk.transpose(1, 0, 2).reshape(128, PAIRS * 128)
    return yt


def kernel(y, Phi, gammas):
    # If tracing is requested but the axon NTFF hook isn't installed in this
    # image, bass_utils would raise ImportError mid-run; degrade to no-trace.
    if os.environ.get("BASS_TRACE"):
        try:
            from antenv.axon_hooks import get_axon_ntff_profile_hook  # noqa
        except ImportError:
            os.environ["BASS_NEVER_TRACE"] = "1"

    from concourse.bass_utils import run_bass_kernel_spmd

    y16 = np.asarray(y, dtype=np.float32).astype(np.float16)
    phi = np.asarray(Phi, dtype=np.float32)
    gam = np.asarray(gammas, dtype=np.float32).reshape(1, STAGES)

    if HOST_M9:
        m9h = _host_m9h(phi, gam)
        consts = {}
    else:
        m9h = np.zeros((128, N), dtype=np.float16)
        bloba = np.zeros((M, AW), dtype=np.float32)
        bloba[:, A_PHI:A_GAM] = phi
        bloba[:, A_GAM:A_I64] = np.broadcast_to(gam, (M, STAGES))
        bloba[:, A_I64:] = np.eye(M, dtype=np.float32)
        blobb = np.zeros((128, BW), dtype=np.float32)
        blobb[:, B_IDH:B_PHI2S] = np.eye(128, dtype=np.float16).view(
            np.float32
        )
        phi2s = (phi / np.float32(SCALE)).astype(np.float32)
        blobb[:M, B_PHI2S:BW] = phi2s
        blobb[M:, B_PHI2S:BW] = phi2s
        consts = {"bloba": bloba, "blobb": blobb}

    nc = _get_nc(HOST_M9)
    in_maps = [
        dict(consts, yt=_pack_yt(y16[i * BS : (i + 1) * BS], m9h))
        for i in range(NCORES)
    ]
    # The runtime occasionally reports a transient "exec unit unrecoverable"
    # fault (~1 in 10 runs, same NEFF passes on retry), so retry a few times.
    last_err = None
    for attempt in range(3):
        try:
            res = run_bass_kernel_spmd(
                nc, in_maps, core_ids=list(range(NCORES))
            )
            break
        except Exception as e:
            last_err = e
            time.sleep(2.0)
    else:
        raise last_err
    _cached["last_run"] = res
    out16 = np.concatenate(
        [res.results[i]["out"] for i in range(NCORES)], axis=0
    )
    return out16.astype(np.float32) * np.float32(SCALE)
